# revision 1
# baseline (speedup 1.0000x reference)
"""GRU encoder step (embedding lookup + GRUCell, batch=1) on 8 TRN2 cores.

Sharding: each core k computes hidden dims [32k, 32k+32) of h_new; the host
concatenates the 8 slices. The embedding table is replicated to every core;
the looked-up row is fetched on-device with an indirect (SWDGE) DMA, so no
collective is needed (an all-gather floor is ~5us, far more than the gather).

Per-core packed operands (HID=256, G=HID//8=32, WA=515):

  a_mat [128, 515] f32 (per-core):
     partitions 0:32=r rows, 32:64=i_n rows, 64:96=z rows, 96:128=h_n rows
     cols 0:256   x-side weights (r, i_n only; z x-weights ship separately)
     cols 256:512 h-side weights
     cols 512,513 biases (paired with the ones columns of bh_mat)
     col 514      partitions 0:32 = hs (this core's h slice, final blend)
  a_zx [32, 256] f32 (per-core): z-row x-side weights at base partition 0
     (kept out of a_mat so a_mat's DMA receipt stays off the critical path)
  bh_mat [128, 258] f32 (shared): cols 0:256 = h row, 256:258 = 1.0
  idx [64, 1] i32 (shared): token index replicated (partition-major — the
     free-dim offset layout hangs on HW)
  table [100000, 256] f32 (shared): full embedding

Device program (straight-line raw bass, manual semaphores, no collectives):
  sync   : idx/a/bh/a_zx DMAs in; result DMA out (HWDGE). No completion
           wait on the output DMA: it lands during the exit barrier.
  gpsimd : one indirect gather with all APs at base partition 0 (base!=0
           offset/dest APs hang on HW): bxa[0:64] = 64 copies of table[idx];
           the z contraction reuses bxa[0:32], so no second gather.
  vector : gh = sum(A_h*Bh) (h-side + biases, fused mul+reduce)
           ghn0 = copy gh[96:128] ; gxa = sum(A_x*bxa) ;
           i_n0 = gxa[32:64]+gh[32:64] ; gxz = sum(a_zx*bxa[0:32]) ;
           p = ghn0*r ; d = hs-n ; out = d*z + n (512B/partition write)
  scalar : ACT-table prewarm ; r = sigmoid(gxa+gh) ; z = sigmoid(gxz+gh_z) ;
           n = tanh(p + bias=i_n0)
"""

import os
import sys

import numpy as np

for _p in ("/opt/trn_rl_repo",):
    if _p not in sys.path and os.path.isdir(_p):
        sys.path.insert(0, _p)

import concourse.bass as bass
from concourse import mybir

VOCAB = 100000
HID = 256
NCORES = 8
G = HID // NCORES  # 32
WA = 2 * HID + 3  # 515

_cached = None


def build_program():
    nc = bass.Bass(
        "TRN2",
        target_bir_lowering=False,
        debug=False,
        enable_asserts=True,
        num_devices=NCORES,
    )
    f32 = mybir.dt.float32
    i32 = mybir.dt.int32

    table = nc.dram_tensor("table", [VOCAB, HID], f32, kind="ExternalInput").ap()
    a_d = nc.dram_tensor("a_mat", [128, WA], f32, kind="ExternalInput").ap()
    azx_d = nc.dram_tensor("a_zx", [G, 2 * HID], f32, kind="ExternalInput").ap()
    bh_d = nc.dram_tensor("bh_mat", [128, HID + 2], f32, kind="ExternalInput").ap()
    idx_d = nc.dram_tensor("idx", [G, 1], i32, kind="ExternalInput").ap()
    # [G, 128] so each partition's DMA write is 512B (avoids sub-512B RMW);
    # host reads column 0.
    out_d = nc.dram_tensor("out", [G, 128], f32, kind="ExternalOutput").ap()

    a_sb = nc.alloc_sbuf_tensor("a_sb", [128, WA], f32).ap()
    azx_sb = nc.alloc_sbuf_tensor("azx_sb", [G, 2 * HID], f32).ap()
    bh_sb = nc.alloc_sbuf_tensor("bh_sb", [128, HID + 2], f32).ap()
    bxa_sb = nc.alloc_sbuf_tensor("bxa_sb", [G, HID], f32).ap()
    idx_sb = nc.alloc_sbuf_tensor("idx_sb", [G, 1], i32).ap()
    s1 = nc.alloc_sbuf_tensor("s1", [G, HID], f32).ap()
    s1b = nc.alloc_sbuf_tensor("s1b", [G, HID], f32).ap()
    s2 = nc.alloc_sbuf_tensor("s2", [128, HID + 2], f32).ap()
    s3 = nc.alloc_sbuf_tensor("s3", [G, HID], f32).ap()
    gh = nc.alloc_sbuf_tensor("gh", [128, 1], f32).ap()
    gxr = nc.alloc_sbuf_tensor("gxr", [G, 1], f32).ap()
    gxi = nc.alloc_sbuf_tensor("gxi", [G, 1], f32).ap()
    gxz = nc.alloc_sbuf_tensor("gxz", [G, 1], f32).ap()
    r_t = nc.alloc_sbuf_tensor("r_t", [G, 1], f32).ap()
    z_t = nc.alloc_sbuf_tensor("z_t", [G, 1], f32).ap()
    in0_t = nc.alloc_sbuf_tensor("in0_t", [G, 1], f32).ap()
    ghn0 = nc.alloc_sbuf_tensor("ghn0", [G, 1], f32).ap()
    p_t = nc.alloc_sbuf_tensor("p_t", [G, 1], f32).ap()
    n_t = nc.alloc_sbuf_tensor("n_t", [G, 1], f32).ap()
    d_t = nc.alloc_sbuf_tensor("d_t", [G, 1], f32).ap()
    out_sb = nc.alloc_sbuf_tensor("out_sb", [G, 128], f32).ap()
    warm = nc.alloc_sbuf_tensor("warm", [G, 1], f32).ap()

    hs_view = a_sb[0:G, 514:515]  # per-core h slice, base partition 0

    with (
        nc.semaphore() as s_idx,
        nc.semaphore() as s_in,
        nc.semaphore() as s_in2,
        nc.semaphore() as s_gx,
        nc.semaphore() as s_v,
        nc.semaphore() as s_ve,
        nc.semaphore() as s_s,
        nc.semaphore() as s_out,
        nc.Block(no_gpsimd_drain=True) as block,
    ):

        @block.sync
        def _(sync):
            sync.dma_start(idx_sb[:], idx_d[:]).then_inc(s_idx, 16)
            sync.dma_start(a_sb[:], a_d[:]).then_inc(s_in, 16)
            sync.dma_start(bh_sb[:], bh_d[:]).then_inc(s_in, 16)
            sync.dma_start(azx_sb[:], azx_d[:]).then_inc(s_in2, 16)
            sync.wait_ge(s_v, 6)
            # No completion wait: engines halt after issue; the DMA lands
            # during the exit barrier / teardown, long before host readback.
            sync.dma_start(out_d[:], out_sb[:]).then_inc(s_out, 16)

        @block.gpsimd
        def _(gpsimd):
            gpsimd.wait_ge(s_idx, 16)
            # single gather: 64 copies of the embedding row (r + i_n rows);
            # the z contraction reuses bxa[0:32], so no second gather.
            gpsimd.indirect_dma_start(
                out=bxa_sb[:],
                out_offset=None,
                in_=table[:],
                in_offset=bass.IndirectOffsetOnAxis(ap=idx_sb[:, :1], axis=0),
            ).then_inc(s_gx, 16)

        @block.vector
        def _(vector):
            vector.wait_ge(s_in, 32)
            # h-side contraction + biases (ones columns): gh = sum(A_h * Bh)
            vector.scalar_tensor_tensor(
                out=s2[:],
                in0=a_sb[:, HID : 2 * HID + 2],
                scalar=1.0,
                in1=bh_sb[:],
                op0=mybir.AluOpType.mult,
                op1=mybir.AluOpType.mult,
                accum_out=gh[:],
            ).then_inc(s_v, 1)
            vector.wait_ge(s_v, 1)  # sem edge for gh (same-engine RAW)
            vector.tensor_copy(out=ghn0[:], in_=gh[3 * G : 4 * G, :1]).then_inc(
                s_ve, 1
            )
            vector.wait_ge(s_gx, 16)
            # x-side contraction, r rows (critical: unlocks sigmoid r)
            vector.scalar_tensor_tensor(
                out=s1[:],
                in0=a_sb[0:G, 0:HID],
                scalar=1.0,
                in1=bxa_sb[:],
                op0=mybir.AluOpType.mult,
                op1=mybir.AluOpType.mult,
                accum_out=gxr[:],
            ).then_inc(s_v, 1)
            # x-side contraction, i_n rows (weights at base 0 in azx cols 0:256)
            vector.wait_ge(s_in2, 16)
            vector.scalar_tensor_tensor(
                out=s1b[:],
                in0=azx_sb[:, 0:HID],
                scalar=1.0,
                in1=bxa_sb[:],
                op0=mybir.AluOpType.mult,
                op1=mybir.AluOpType.mult,
                accum_out=gxi[:],
            ).then_inc(s_v, 1)
            # x-side contraction, z rows (azx cols 256:512)
            vector.scalar_tensor_tensor(
                out=s3[:],
                in0=azx_sb[:, HID : 2 * HID],
                scalar=1.0,
                in1=bxa_sb[:],
                op0=mybir.AluOpType.mult,
                op1=mybir.AluOpType.mult,
                accum_out=gxz[:],
            ).then_inc(s_v, 1)
            vector.wait_ge(s_s, 1)  # r_t ready
            vector.wait_ge(s_ve, 1)  # sem edge for ghn0
            vector.tensor_tensor(
                out=p_t[:], in0=ghn0[:], in1=r_t[:], op=mybir.AluOpType.mult
            ).then_inc(s_v, 1)
            vector.wait_ge(s_s, 3)  # n ready (tanh is 3rd inc)
            vector.tensor_tensor(
                out=d_t[:], in0=hs_view, in1=n_t[:], op=mybir.AluOpType.subtract
            ).then_inc(s_ve, 1)
            vector.wait_ge(s_s, 4)  # z_t ready
            vector.wait_ge(s_ve, 2)  # sem edge for d_t (same-engine RAW)
            vector.scalar_tensor_tensor(
                out=out_sb[:],
                in0=d_t[:, :1].to_broadcast([G, 128]),
                scalar=z_t[:, :1],
                in1=n_t[:, :1].to_broadcast([G, 128]),
                op0=mybir.AluOpType.mult,
                op1=mybir.AluOpType.add,
            ).then_inc(s_v, 1)

        @block.scalar
        def _(scalar):
            # Warm the ACT function table while DMAs/gather are in flight —
            # otherwise a ~1.3us ACT_TABLE_LOAD lands on the critical path.
            const0 = nc.const_aps.aps[(f32, 0.0)]
            scalar.activation(
                warm[:], const0[0:G, :1], mybir.ActivationFunctionType.Sigmoid
            )
            scalar.wait_ge(s_v, 2)  # gh and gxr ready
            scalar.activation(
                r_t[:],
                gxr[:, :1],
                mybir.ActivationFunctionType.Sigmoid,
                bias=gh[0:G, :1],
            ).then_inc(s_s, 1)
            scalar.wait_ge(s_v, 3)  # gxi ready
            scalar.activation(
                in0_t[:],
                gxi[:, :1],
                mybir.ActivationFunctionType.Identity,
                bias=gh[G : 2 * G, :1],
            ).then_inc(s_s, 1)
            scalar.wait_ge(s_v, 5)  # p_t ready
            scalar.wait_ge(s_s, 2)  # sem edge for in0_t (same-engine RAW)
            scalar.activation(
                n_t[:],
                p_t[:],
                mybir.ActivationFunctionType.Tanh,
                bias=in0_t[:, :1],
            ).then_inc(s_s, 1)
            scalar.wait_ge(s_v, 4)  # gxz ready
            scalar.activation(
                z_t[:],
                gxz[:, :1],
                mybir.ActivationFunctionType.Sigmoid,
                bias=gh[2 * G : 3 * G, :1],
            ).then_inc(s_s, 1)

    return nc


def shard_inputs(
    input, hidden, embedding, w_ih, w_hh, b_ih, b_hh
) -> list[dict[str, np.ndarray]]:
    """Host-side marshaling: slice/replicate full inputs into per-core maps."""
    idx = int(np.asarray(input).reshape(-1)[0])
    h = np.asarray(hidden, dtype=np.float32).reshape(HID)
    table = np.ascontiguousarray(np.asarray(embedding, dtype=np.float32))
    w_ih = np.asarray(w_ih, dtype=np.float32)
    w_hh = np.asarray(w_hh, dtype=np.float32)
    b_ih = np.asarray(b_ih, dtype=np.float32)
    b_hh = np.asarray(b_hh, dtype=np.float32)

    idx_arr = np.full((G, 1), idx, dtype=np.int32)
    bh = np.empty((128, HID + 2), dtype=np.float32)
    bh[:, 0:HID] = h[None, :]
    bh[:, HID:] = 1.0

    in_maps = []
    for k in range(NCORES):
        lo = G * k
        a = np.zeros((128, WA), dtype=np.float32)
        # r rows: partitions 0:32
        a[0:G, 0:HID] = w_ih[lo : lo + G]
        a[0:G, HID : 2 * HID] = w_hh[lo : lo + G]
        a[0:G, 2 * HID] = b_ih[lo : lo + G]
        a[0:G, 2 * HID + 1] = b_hh[lo : lo + G]
        # i_n rows: partitions 32:64 (x side only)
        a[G : 2 * G, 0:HID] = w_ih[2 * HID + lo : 2 * HID + lo + G]
        a[G : 2 * G, 2 * HID] = b_ih[2 * HID + lo : 2 * HID + lo + G]
        # z rows: h-side on partitions 64:96, x-side on partitions 0:32
        # at cols 515:771
        a[2 * G : 3 * G, HID : 2 * HID] = w_hh[HID + lo : HID + lo + G]
        a[2 * G : 3 * G, 2 * HID] = b_ih[HID + lo : HID + lo + G]
        a[2 * G : 3 * G, 2 * HID + 1] = b_hh[HID + lo : HID + lo + G]
        azx = np.concatenate(
            [w_ih[2 * HID + lo : 2 * HID + lo + G], w_ih[HID + lo : HID + lo + G]],
            axis=1,
        )
        # h_n rows: partitions 96:128 (h side only)
        a[3 * G : 4 * G, HID : 2 * HID] = w_hh[2 * HID + lo : 2 * HID + lo + G]
        a[3 * G : 4 * G, 2 * HID] = b_hh[2 * HID + lo : 2 * HID + lo + G]
        # hs column (h slice for the final blend)
        a[0:G, 2 * HID + 2] = h[lo : lo + G]

        in_maps.append(
            {"table": table, "a_mat": a, "a_zx": azx, "bh_mat": bh, "idx": idx_arr}
        )
    return in_maps


def unshard_output(results: list[dict[str, np.ndarray]]):
    h_new = np.concatenate(
        [np.asarray(results[k]["out"]).reshape(G, -1)[:, 0] for k in range(NCORES)]
    ).astype(np.float32)
    out = h_new.reshape(1, 1, HID)
    return out, out


def _get_program():
    global _cached
    if _cached is None:
        _cached = build_program()
    return _cached


def kernel(**inputs):
    from concourse.bass_utils import run_bass_kernel_spmd

    nc = _get_program()
    in_maps = shard_inputs(**inputs)
    res = run_bass_kernel_spmd(nc, in_maps, core_ids=list(range(NCORES)))
    return unshard_output(res.results)


def run_traced(**inputs):
    """Like kernel() but with NTFF tracing; returns (output, BassKernelResults)."""
    from concourse.bass_utils import run_bass_kernel_spmd

    nc = _get_program()
    in_maps = shard_inputs(**inputs)
    res = run_bass_kernel_spmd(nc, in_maps, core_ids=list(range(NCORES)), trace=True)
    return unshard_output(res.results), res



# revision 4
# speedup vs baseline: 1.2782x; 1.2782x over previous
"""GRU encoder step (embedding lookup + GRUCell, batch=1) on 8 TRN2 cores.

Sharding: core k computes hidden dims [32k, 32k+32) of h_new; the host
concatenates the 8 slices. The embedding table is replicated to every core.

The lookup avoids SWDGE entirely: the Sync engine reg_loads the token index
straight from DRAM into a register (~1.1us), then issues a regular HWDGE DMA
of table[ds(idx,1),:] with a stride-0 source broadcast that writes the row
to 96 SBUF partitions. This replaces the baseline's idx-DMA -> gpsimd
indirect-gather chain (~6.4us) with ~3.6us.

Per-core packed operand wb [128, 775] f32 (one DMA, issued by Scalar):
  partition map: p0:32 = r gate rows, p32:64 = i_n rows, p64:96 = z rows,
                 p96:128 = h_n rows (w_ih rows: r=0:256, z=256:512, n=512:768)
  cols 0:256    x-side weights (rows r/i_n/z; p96:128 zero)
  col 256       b_ih (r/i_n/z rows)
  col 257       b_hh (r and z rows only; b_hh[n] belongs inside r*(.))
  cols 258:514  h-side weights (r, zero, z, h_n)
  col 514       b_hh[n] on p96:128 (pairs with ones col 773)
  col 515       zero (pairs with ones col 774)
  col 516       hs = this core's h slice on p64:96 (for q = z*hs)
  cols 517:773  h replicated on all 128 partitions
  cols 773:775  1.0 (ones pair for the gh bias columns)

x_sb [96, 258]: cols 0:256 gathered row (broadcast DMA), cols 256:258 ones
  (vector writes them, pairs with wb cols 256:258 bias pair).

Contractions (vector, fused mul+reduce):
  gh[128] = sum(wb[:,258:516] * wb[:,517:775])   -> Wh.h (+b_hn on p96:128)
  gx[96]  = sum(wb[0:96,0:258] * x_sb)           -> Wx.x + b_ih (+b_hh r/z)
Scalar: rz = sigmoid(gx[0:96] + gh[0:96]) (r at p0:32, z at p64:96; p32:64
  is a harmless garbage lane), then n = tanh(rz_r * gh[96:128] + gx[32:64])
  using the activation scale/bias APs (i_n needs no add: gh[32:64] == 0).
Vector: u = 1-z, q = z*hs (at p64:96), out = n*u + q -> out_sb[32,1].
Sync: waits, DMAs out_sb to DRAM (lands during teardown).
"""

import os
import sys

import numpy as np

for _p in ("/opt/trn_rl_repo",):
    if _p not in sys.path and os.path.isdir(_p):
        sys.path.insert(0, _p)

import concourse.bass as bass
from concourse import mybir

VOCAB = 100000
HID = 256
NCORES = 8
G = HID // NCORES  # 32
WB_W = 775

_cached = None


def build_program():
    nc = bass.Bass(
        "TRN2",
        target_bir_lowering=False,
        debug=False,
        num_devices=NCORES,
    )
    f32 = mybir.dt.float32
    i32 = mybir.dt.int32

    table = nc.dram_tensor("table", [VOCAB, HID], f32, kind="ExternalInput").ap()
    wb_d = nc.dram_tensor("wb", [128, WB_W], f32, kind="ExternalInput").ap()
    idx_d = nc.dram_tensor("idx", [1, 1], i32, kind="ExternalInput").ap()
    out_d = nc.dram_tensor("out", [G, 1], f32, kind="ExternalOutput").ap()

    wb_sb = nc.alloc_sbuf_tensor("wb_sb", [128, WB_W], f32).ap()
    x_sb = nc.alloc_sbuf_tensor("x_sb", [96, HID + 2], f32).ap()
    s1 = nc.alloc_sbuf_tensor("s1", [96, HID + 2], f32).ap()
    s2 = nc.alloc_sbuf_tensor("s2", [128, HID + 2], f32).ap()
    gx = nc.alloc_sbuf_tensor("gx", [96, 1], f32).ap()
    gh = nc.alloc_sbuf_tensor("gh", [128, 1], f32).ap()
    rz_t = nc.alloc_sbuf_tensor("rz_t", [96, 1], f32).ap()
    n_t = nc.alloc_sbuf_tensor("n_t", [G, 1], f32).ap()
    u_t = nc.alloc_sbuf_tensor("u_t", [G, 1], f32).ap()
    q_t = nc.alloc_sbuf_tensor("q_t", [G, 1], f32).ap()
    out_sb = nc.alloc_sbuf_tensor("out_sb", [G, 1], f32).ap()
    warm = nc.alloc_sbuf_tensor("warm", [G, 1], f32).ap()

    with (
        nc.semaphore() as s_w,
        nc.semaphore() as s_x,
        nc.semaphore() as s_v,
        nc.semaphore() as s_s,
        nc.semaphore() as s_o,
        nc.Block() as block,
    ):

        @block.sync
        def _(sync):
            with sync.register("ridx") as ridx:
                sync.reg_load(ridx, idx_d[0:1, 0:1])
                off = sync.snap(ridx, min_val=0, max_val=VOCAB - 1)
                sync.dma_start(
                    x_sb[0:96, 0:HID],
                    table[bass.ds(off, 1), :].to_broadcast([96, HID]),
                ).then_inc(s_x, 16)
            sync.wait_ge(s_v, 6)
            # No completion wait: lands during the exit barrier/teardown.
            sync.dma_start(out_d[:], out_sb[:]).then_inc(s_o, 16)

        @block.vector
        def _(vector):
            ones = nc.const_aps.aps[(f32, 1.0)]
            vector.tensor_copy(
                out=x_sb[0:96, HID : HID + 2],
                in_=ones[0:96, :1].to_broadcast([96, 2]),
            ).then_inc(s_v, 1)
            vector.wait_ge(s_w, 16)
            # h-side contraction (+ b_hn via ones cols): gh = sum(Wh * h)
            vector.scalar_tensor_tensor(
                out=s2[:],
                in0=wb_sb[:, 258 : 258 + HID + 2],
                scalar=1.0,
                in1=wb_sb[:, 517 : 517 + HID + 2],
                op0=mybir.AluOpType.mult,
                op1=mybir.AluOpType.mult,
                accum_out=gh[:],
            ).then_inc(s_v, 1)
            vector.wait_ge(s_x, 16)
            vector.wait_ge(s_v, 1)  # sem edge for ones cols (same-engine RAW)
            # x-side contraction for r/i_n/z rows (+ biases via ones cols)
            vector.scalar_tensor_tensor(
                out=s1[:],
                in0=wb_sb[0:96, 0 : HID + 2],
                scalar=1.0,
                in1=x_sb[:],
                op0=mybir.AluOpType.mult,
                op1=mybir.AluOpType.mult,
                accum_out=gx[:],
            ).then_inc(s_v, 1)
            vector.wait_ge(s_s, 1)  # rz ready (z at p64:96)
            vector.scalar_tensor_tensor(
                out=u_t[0:G],
                in0=rz_t[64:96],
                scalar=-1.0,
                in1=x_sb[64:96, HID : HID + 1],
                op0=mybir.AluOpType.mult,
                op1=mybir.AluOpType.add,
            ).then_inc(s_v, 1)
            vector.tensor_tensor(
                out=q_t[0:G],
                in0=rz_t[64:96],
                in1=wb_sb[64:96, 516:517],
                op=mybir.AluOpType.mult,
            ).then_inc(s_v, 1)
            vector.wait_ge(s_s, 2)  # n ready
            vector.wait_ge(s_v, 5)  # sem edge for u_t/q_t (same-engine RAW)
            vector.scalar_tensor_tensor(
                out=out_sb[:],
                in0=n_t[:],
                scalar=u_t[0:G, :1],
                in1=q_t[0:G, :1],
                op0=mybir.AluOpType.mult,
                op1=mybir.AluOpType.add,
            ).then_inc(s_v, 1)

        @block.scalar
        def _(scalar):
            scalar.dma_start(wb_sb[:], wb_d[:]).then_inc(s_w, 16)
            # Warm the ACT table while DMAs are in flight.
            const0 = nc.const_aps.aps[(f32, 0.0)]
            scalar.activation(
                warm[:], const0[0:G, :1], mybir.ActivationFunctionType.Sigmoid
            )
            scalar.wait_ge(s_v, 3)  # gh and gx ready
            # r and z sigmoids in one shot (p32:64 lane is unused garbage)
            scalar.activation(
                rz_t[:],
                gx[:, :1],
                mybir.ActivationFunctionType.Sigmoid,
                bias=gh[0:96, :1],
            ).then_inc(s_s, 1)
            scalar.wait_ge(s_s, 1)  # sem edge for rz_t (same-engine RAW)
            # n = tanh(r * ghn + i_n); gh[32:64]==0 so bias=gx[32:64] is i_n
            scalar.activation(
                n_t[:],
                rz_t[0:G, :1],
                mybir.ActivationFunctionType.Tanh,
                scale=gh[96:128, :1],
                bias=gx[G : 2 * G, :1],
            ).then_inc(s_s, 1)

    return nc


def shard_inputs(
    input, hidden, embedding, w_ih, w_hh, b_ih, b_hh
) -> list[dict[str, np.ndarray]]:
    """Host-side marshaling: slice/replicate full inputs into per-core maps."""
    idx = int(np.asarray(input).reshape(-1)[0])
    h = np.asarray(hidden, dtype=np.float32).reshape(HID)
    table = np.ascontiguousarray(np.asarray(embedding, dtype=np.float32))
    w_ih = np.asarray(w_ih, dtype=np.float32)
    w_hh = np.asarray(w_hh, dtype=np.float32)
    b_ih = np.asarray(b_ih, dtype=np.float32)
    b_hh = np.asarray(b_hh, dtype=np.float32)

    idx_arr = np.full((1, 1), idx, dtype=np.int32)

    in_maps = []
    for k in range(NCORES):
        lo = G * k
        r_sl = slice(lo, lo + G)
        z_sl = slice(HID + lo, HID + lo + G)
        n_sl = slice(2 * HID + lo, 2 * HID + lo + G)
        wb = np.zeros((128, WB_W), dtype=np.float32)
        # x-side weights
        wb[0:G, 0:HID] = w_ih[r_sl]
        wb[G : 2 * G, 0:HID] = w_ih[n_sl]
        wb[2 * G : 3 * G, 0:HID] = w_ih[z_sl]
        # x-side biases (col 256 = b_ih, col 257 = b_hh where it belongs)
        wb[0:G, HID] = b_ih[r_sl]
        wb[G : 2 * G, HID] = b_ih[n_sl]
        wb[2 * G : 3 * G, HID] = b_ih[z_sl]
        wb[0:G, HID + 1] = b_hh[r_sl]
        wb[2 * G : 3 * G, HID + 1] = b_hh[z_sl]
        # h-side weights
        wb[0:G, 258 : 258 + HID] = w_hh[r_sl]
        wb[2 * G : 3 * G, 258 : 258 + HID] = w_hh[z_sl]
        wb[3 * G : 4 * G, 258 : 258 + HID] = w_hh[n_sl]
        # gh bias: b_hh[n] rows pair with ones col 773
        wb[3 * G : 4 * G, 514] = b_hh[n_sl]
        # hs on z partitions (for q = z*hs)
        wb[2 * G : 3 * G, 516] = h[r_sl]
        # h replicated + ones pair
        wb[:, 517 : 517 + HID] = h[None, :]
        wb[:, 773:775] = 1.0

        in_maps.append({"table": table, "wb": wb, "idx": idx_arr})
    return in_maps


def unshard_output(results: list[dict[str, np.ndarray]]):
    h_new = np.concatenate(
        [np.asarray(results[k]["out"]).reshape(G) for k in range(NCORES)]
    ).astype(np.float32)
    out = h_new.reshape(1, 1, HID)
    return out, out


def _get_program():
    global _cached
    if _cached is None:
        _cached = build_program()
    return _cached


def kernel(**inputs):
    from concourse.bass_utils import run_bass_kernel_spmd

    nc = _get_program()
    in_maps = shard_inputs(**inputs)
    res = run_bass_kernel_spmd(nc, in_maps, core_ids=list(range(NCORES)))
    return unshard_output(res.results)


def run_traced(**inputs):
    """Like kernel() but with NTFF tracing; returns (output, BassKernelResults)."""
    from concourse.bass_utils import run_bass_kernel_spmd

    nc = _get_program()
    in_maps = shard_inputs(**inputs)
    res = run_bass_kernel_spmd(nc, in_maps, core_ids=list(range(NCORES)), trace=True)
    return unshard_output(res.results), res
